# revision 1
# baseline (speedup 1.0000x reference)
"""Self-contained Trainium2 Bass kernel for nn_MixtureOfExperts_515396075673.

MoE: T=4096 tokens, D=1024, H=2048, E=8 experts, top-2, SwiGLU.

Strategy (expert-parallel, routed):
  - 8 NeuronCores, one expert per core; router replicated in fp32 on every
    core (top-2 selection gaps can be ~1e-5, so the router must be fp32).
  - On device per core: router logits (fp32 PE) -> top-2 mask + renormalized
    gate -> compaction of the tokens routed to this core's expert (prefix
    scan + triangular-matmul cross-partition prefix + indirect scatter of
    (token_id, gate) pairs) -> indirect row gather of selected tokens (bf16)
    -> SwiGLU expert in bf16 (weights resident in SBUF) -> gate scaling ->
    compacted output yT [D, C] plus the token index list.
  - Host: scatter-add the 8 compacted outputs into the full [T, D] output
    (each token is routed to exactly 2 experts; padding slots carry gate 0
    and token id 0, contributing exactly zero).
"""
import sys
sys.path.insert(0, "/opt/trn_rl_repo")

import numpy as np
import ml_dtypes
import concourse.bass as bass
import concourse.mybir as mybir
import concourse.tile as tile
from concourse import bacc
from concourse.bass import IndirectOffsetOnAxis
from concourse.bass_utils import run_bass_kernel_spmd

FP32 = mybir.dt.float32
F32R = mybir.dt.float32r
BF16 = mybir.dt.bfloat16
I32 = mybir.dt.int32

T = 4096          # tokens
D = 1024          # model dim
H = 2048          # hidden
E = 8             # experts
P = 128           # partitions
BIG = 100000.0


def build_moe_program(n_iters=1, expert_dtype="bf16", C=1280, silu_mode="sigmoid"):
    """Build the (uncompiled) Bacc program. Returns nc."""
    assert expert_dtype == "bf16"
    KD = D // P       # 8 k-chunks over model dim
    KH = H // P       # 16 k-chunks over hidden dim
    NTT = T // P      # 32 token tiles
    G = 16            # token tiles per router group
    NG = NTT // G
    nt_sizes = []
    rem = C
    while rem > 0:
        s = min(512, rem)
        nt_sizes.append(s)
        rem -= s
    wdt = BF16

    nc = bacc.Bacc("TRN2", target_bir_lowering=False, debug=False, num_devices=8)

    # ---- DRAM I/O ----
    xTr = nc.dram_tensor("xTr", [T // P, P, D // P, P], FP32, kind="ExternalInput").ap()
    xbf = nc.dram_tensor("xbf", [T + 1, D], BF16, kind="ExternalInput").ap()
    Wr = nc.dram_tensor("Wr", [D, E], FP32, kind="ExternalInput").ap()
    tri = nc.dram_tensor("tri", [P, P], FP32, kind="ExternalInput").ap()
    onehot = nc.dram_tensor("onehot", [P, E], FP32, kind="ExternalInput").ap()
    ident = nc.dram_tensor("ident", [P, P], BF16, kind="ExternalInput").ap()
    idx_init = nc.dram_tensor("idx_init", [C + 1, 2], I32, kind="ExternalInput").ap()
    w1 = nc.dram_tensor("w1", [D, H], wdt, kind="ExternalInput").ap()
    w3 = nc.dram_tensor("w3", [D, H], wdt, kind="ExternalInput").ap()
    w2 = nc.dram_tensor("w2", [H, D], wdt, kind="ExternalInput").ap()

    yT_out = nc.dram_tensor("yT_out", [D, C], FP32, kind="ExternalOutput").ap()
    idx_out = nc.dram_tensor("idx_out", [C + 1, 1], I32, kind="ExternalOutput").ap()

    with tile.TileContext(nc) as tc:
        def body():
            with (
                tc.tile_pool(name="const", bufs=1) as constp,
                tc.tile_pool(name="dram", bufs=1, space="DRAM") as dramp,
            ):
                # ---- constants + resident weights ----
                Wr_sb = constp.tile([P, KD, E], FP32)
                nc.sync.dma_start(Wr_sb[:], Wr.rearrange("(k p) e -> p k e", p=P))
                tri_sb = constp.tile([P, P], FP32)
                nc.sync.dma_start(tri_sb[:], tri[:])
                oh_sb = constp.tile([P, E], FP32)
                nc.sync.dma_start(oh_sb[:], onehot[:])
                id_sb = constp.tile([P, P], BF16)
                nc.sync.dma_start(id_sb[:], ident[:])
                w1sb = constp.tile([P, KD, H], wdt)
                nc.sync.dma_start(w1sb[:], w1.rearrange("(k p) h -> p k h", p=P))
                w3sb = constp.tile([P, KD, H], wdt)
                nc.sync.dma_start(w3sb[:], w3.rearrange("(k p) h -> p k h", p=P))
                w2sb = constp.tile([P, KH, D], wdt)
                nc.sync.dma_start(w2sb[:], w2.rearrange("(k p) d -> p k d", p=P))

                ig_dram = dramp.tile([C + 1, 2], I32, name="ig_dram")
                gate_all = constp.tile([P, NTT], FP32)
                mask_all = constp.tile([P, NTT], FP32)

                # ======== router ========
                with (
                    tc.tile_pool(name="rt_sb", bufs=2) as rtp,
                    tc.tile_pool(name="rt_ps", bufs=2, space="PSUM") as rtps,
                ):
                    ig = constp.tile([P, NTT * 2], I32, name="ig")
                    ig3 = ig.rearrange("p (i two) -> p i two", two=2)
                    nc.gpsimd.iota(ig3[:, :, 0], pattern=[[P, NTT]], base=0,
                                   channel_multiplier=1)
                    nc.sync.dma_start(ig_dram[:], idx_init[:])
                    base_bc = constp.tile([P, 1], FP32, name="base_bc")
                    nc.vector.memset(base_bc[:], 0.0)
                    for g in range(NG):
                        psum_l = rtps.tile([P, G * E], FP32, tag="psl")
                        for t in range(G):
                            tt = g * G + t
                            xr = rtp.tile([P, KD, P], FP32, tag="xr", bufs=8)
                            nc.sync.dma_start(xr[:], xTr[tt])
                            for kc in range(KD):
                                nc.tensor.matmul(
                                    psum_l[:, t * E:(t + 1) * E],
                                    lhsT=xr[:, kc, :],
                                    rhs=Wr_sb[:, kc, :],
                                    start=(kc == 0), stop=(kc == KD - 1))
                        L3 = psum_l.rearrange("p (t e) -> p t e", e=E)
                        m1 = rtp.tile([P, G], FP32, tag="m1")
                        nc.vector.reduce_max(m1[:, :, None], L3, axis=mybir.AxisListType.X)
                        eq = rtp.tile([P, G * E], FP32, tag="eq")
                        eq3 = eq.rearrange("p (t e) -> p t e", e=E)
                        nc.vector.tensor_tensor(
                            out=eq3, in0=L3, in1=m1[:, :, None].to_broadcast((P, G, E)),
                            op=mybir.AluOpType.is_equal)
                        lm = rtp.tile([P, G * E], FP32, tag="lm")
                        nc.vector.tensor_scalar_mul(lm[:], eq[:], -1e30)
                        lm3 = lm.rearrange("p (t e) -> p t e", e=E)
                        nc.vector.tensor_tensor(out=lm3, in0=lm3, in1=L3,
                                                op=mybir.AluOpType.add)
                        m2 = rtp.tile([P, G], FP32, tag="m2")
                        nc.vector.reduce_max(m2[:, :, None], lm3, axis=mybir.AxisListType.X)
                        zs = rtp.tile([P, G * E], FP32, tag="zs")
                        zs3 = zs.rearrange("p (t e) -> p t e", e=E)
                        nc.vector.tensor_tensor(
                            out=zs3, in0=L3, in1=m1[:, :, None].to_broadcast((P, G, E)),
                            op=mybir.AluOpType.subtract)
                        nc.scalar.activation(zs[:], zs[:], mybir.ActivationFunctionType.Exp)
                        em = rtp.tile([P, G], FP32, tag="em")
                        nc.vector.tensor_tensor(out=em[:], in0=m2[:], in1=m1[:],
                                                op=mybir.AluOpType.subtract)
                        nc.scalar.activation(em[:], em[:], mybir.ActivationFunctionType.Exp)
                        den = rtp.tile([P, G], FP32, tag="den")
                        nc.vector.tensor_scalar_add(den[:], em[:], 1.0)
                        rden = rtp.tile([P, G], FP32, tag="rden")
                        nc.vector.reciprocal(rden[:], den[:])
                        sel = rtp.tile([P, G * E], FP32, tag="sel")
                        sel3 = sel.rearrange("p (t e) -> p t e", e=E)
                        nc.vector.tensor_tensor(
                            out=sel3, in0=L3, in1=m2[:, :, None].to_broadcast((P, G, E)),
                            op=mybir.AluOpType.is_ge)
                        gt = rtp.tile([P, G * E], FP32, tag="gt")
                        nc.vector.tensor_tensor(out=gt[:], in0=zs[:], in1=sel[:],
                                                op=mybir.AluOpType.mult)
                        gt3 = gt.rearrange("p (t e) -> p t e", e=E)
                        nc.vector.tensor_tensor(
                            out=gt3, in0=gt3, in1=rden[:, :, None].to_broadcast((P, G, E)),
                            op=mybir.AluOpType.mult)
                        nc.vector.tensor_tensor(
                            out=gt3, in0=gt3, in1=oh_sb[:, None, :].to_broadcast((P, G, E)),
                            op=mybir.AluOpType.mult)
                        nc.vector.reduce_sum(
                            gate_all[:, g * G:(g + 1) * G][:, :, None], gt3,
                            axis=mybir.AxisListType.X)
                        nc.vector.tensor_tensor(
                            out=sel3, in0=sel3, in1=oh_sb[:, None, :].to_broadcast((P, G, E)),
                            op=mybir.AluOpType.mult)
                        nc.vector.reduce_sum(
                            mask_all[:, g * G:(g + 1) * G][:, :, None], sel3,
                            axis=mybir.AxisListType.X)

                        # ---- per-group compaction + scatter ----
                        nc.vector.tensor_copy(
                            ig3[:, g * G:(g + 1) * G, 1].bitcast(FP32),
                            gate_all[:, g * G:(g + 1) * G])
                        maskg = mask_all[:, g * G:(g + 1) * G]
                        incl = rtp.tile([P, G], FP32, tag="incl")
                        nc.vector.tensor_tensor_scan(
                            out=incl[:], data0=maskg, data1=maskg,
                            initial=0.0, op0=mybir.AluOpType.add,
                            op1=mybir.AluOpType.bypass)
                        excl = rtp.tile([P, G], FP32, tag="excl")
                        nc.vector.tensor_tensor(out=excl[:], in0=incl[:], in1=maskg,
                                                op=mybir.AluOpType.subtract)
                        tot = rtp.tile([P, 1], FP32, tag="tot")
                        nc.vector.tensor_copy(tot[:], incl[:, G - 1:G])
                        ps_off = rtps.tile([P, 1], FP32, tag="psoff")
                        nc.tensor.matmul(ps_off[:], lhsT=tri_sb[:], rhs=tot[:],
                                         start=True, stop=True)
                        pos = rtp.tile([P, G], FP32, tag="pos")
                        nc.vector.tensor_scalar_add(pos[:], excl[:], ps_off[:, 0:1])
                        nc.vector.tensor_scalar_add(pos[:], pos[:], base_bc[:, 0:1])
                        pm = rtp.tile([P, G], FP32, tag="pm")
                        nc.vector.tensor_scalar(
                            out=pm[:], in0=maskg, scalar1=-BIG, scalar2=BIG,
                            op0=mybir.AluOpType.mult, op1=mybir.AluOpType.add)
                        nc.vector.tensor_tensor(out=pm[:], in0=pm[:], in1=pos[:],
                                                op=mybir.AluOpType.add)
                        posi = rtp.tile([P, G], I32, tag="posi")
                        nc.vector.tensor_copy(posi[:], pm[:])
                        # update base: base += ps_off[127] + tot[127]
                        if g < NG - 1:
                            nb = rtp.tile([1, 1], FP32, tag="nb")
                            nc.gpsimd.tensor_reduce(
                                nb[:], tot[:], axis=mybir.AxisListType.C,
                                op=mybir.AluOpType.add)
                            nc.vector.tensor_tensor(
                                out=nb[:], in0=nb[:], in1=base_bc[0:1, 0:1],
                                op=mybir.AluOpType.add)
                            nc.gpsimd.partition_broadcast(base_bc[:], nb[:])
                        if True:
                            for il in range(G):
                                i = g * G + il
                                nc.gpsimd.indirect_dma_start(
                                    out=ig_dram[:],
                                    out_offset=IndirectOffsetOnAxis(
                                        ap=posi[:, il:il + 1], axis=0),
                                    in_=ig[:, 2 * i:2 * i + 2], in_offset=None,
                                    bounds_check=C, oob_is_err=False)
                    nc.sync.dma_start(idx_out[:], ig_dram[:, 0:1])

                    nc.sync.dma_start(idx_out[:], ig_dram[:, 0:1])

                # ======== gather ========
                NJ = C // P
                xgT = constp.tile([P, KD, C], wdt, name="xgT")
                with (
                    tc.tile_pool(name="ga", bufs=3) as gap,
                    tc.tile_pool(name="ga_ps", bufs=4, space="PSUM") as gaps,
                ):
                    idx_sb = gap.tile([P, NJ], I32, tag="idx")
                    nc.sync.dma_start(
                        idx_sb[:],
                        ig_dram[0:C, 0:1].rearrange("(j p) o -> p j o", p=P)[:, :, 0])
                    gate_row = constp.tile([1, C], FP32, name="gate_row")
                    nc.sync.dma_start(
                        gate_row[:],
                        ig_dram[0:C, 1:2].bitcast(FP32).rearrange("q o -> o q"))
                    for j in range(NJ):
                        xg = gap.tile([P, D], wdt, tag="xg")
                        nc.gpsimd.indirect_dma_start(
                            out=xg[:], out_offset=None,
                            in_=xbf[:],
                            in_offset=IndirectOffsetOnAxis(ap=idx_sb[:, j:j + 1], axis=0))
                        for dd in range(KD):
                            tp = gaps.tile([P, P], wdt, tag="tp")
                            nc.tensor.transpose(tp[:], xg[:, dd * P:(dd + 1) * P], id_sb[:])
                            nc.vector.tensor_copy(xgT[:, dd, j * P:(j + 1) * P], tp[:])
                gate_bc = constp.tile([P, C], FP32)
                nc.gpsimd.partition_broadcast(gate_bc[:], gate_row[:])

                # ======== expert SwiGLU ========
                hT = [constp.tile([P, C], wdt, name=f"hT{m}") for m in range(KH)]
                with (
                    tc.tile_pool(name="ex_ps", bufs=2, space="PSUM") as exps,
                    tc.tile_pool(name="ex_sb", bufs=3) as exsb,
                ):
                    for i, s in enumerate(nt_sizes):
                        o = sum(nt_sizes[:i])
                        for mc in range(KH):
                            ph1 = exps.tile([P, s], FP32, tag="ph1", name="ph1")
                            for kc in range(KD):
                                nc.tensor.matmul(
                                    ph1[:], lhsT=w1sb[:, kc, mc * P:(mc + 1) * P],
                                    rhs=xgT[:, kc, o:o + s],
                                    start=(kc == 0), stop=(kc == KD - 1))
                            ph3 = exps.tile([P, s], FP32, tag="ph3", name="ph3")
                            for kc in range(KD):
                                nc.tensor.matmul(
                                    ph3[:], lhsT=w3sb[:, kc, mc * P:(mc + 1) * P],
                                    rhs=xgT[:, kc, o:o + s],
                                    start=(kc == 0), stop=(kc == KD - 1))
                            sg = exsb.tile([P, s], wdt, tag="sg", name="sg")
                            if silu_mode == "act":
                                nc.scalar.activation(sg[:], ph1[:],
                                                     mybir.ActivationFunctionType.Silu)
                                nc.vector.tensor_tensor(
                                    out=hT[mc][:, o:o + s], in0=sg[:], in1=ph3[:],
                                    op=mybir.AluOpType.mult)
                            else:
                                nc.scalar.activation(sg[:], ph1[:],
                                                     mybir.ActivationFunctionType.Sigmoid)
                                nc.vector.tensor_tensor(
                                    out=sg[:], in0=sg[:], in1=ph3[:],
                                    op=mybir.AluOpType.mult)
                                nc.vector.tensor_tensor(
                                    out=hT[mc][:, o:o + s], in0=sg[:], in1=ph1[:],
                                    op=mybir.AluOpType.mult)
                    for i, s in enumerate(nt_sizes):
                        o = sum(nt_sizes[:i])
                        for dc in range(KD):
                            py = exps.tile([P, s], FP32, tag="py", name="py")
                            for hc in range(KH):
                                nc.tensor.matmul(
                                    py[:], lhsT=w2sb[:, hc, dc * P:(dc + 1) * P],
                                    rhs=hT[hc][:, o:o + s],
                                    start=(hc == 0), stop=(hc == KH - 1))
                            ys = exsb.tile([P, s], FP32, tag="ys", name="ys")
                            nc.vector.tensor_tensor(
                                out=ys[:], in0=py[:], in1=gate_bc[:, o:o + s],
                                op=mybir.AluOpType.mult)
                            nc.sync.dma_start(
                                yT_out[dc * P:(dc + 1) * P, o:o + s], ys[:])

        if n_iters == 1:
            body()
        else:
            with tc.For_i(0, n_iters, 1):
                body()

    nc.compile()
    return nc


# ---------------- host side ----------------

def host_prepare(x, Wr, W1, W2, W3, expert_dtype="bf16", C=1280):
    """Build the 8 per-core input maps."""
    xf = np.ascontiguousarray(x.reshape(T, D).astype(np.float32))
    # [NTT, P, KD, P]: xTr[tt, p, k, n] = x[tt*128+n, k*128+p]
    xTr_np = np.ascontiguousarray(
        xf.reshape(T // P, P, D // P, P).transpose(0, 3, 2, 1))
    xbf_np = np.zeros((T + 1, D), ml_dtypes.bfloat16)
    xbf_np[:T] = xf.astype(ml_dtypes.bfloat16)
    tri_np = np.triu(np.ones((P, P), np.float32), 1)
    idx_init_np = np.zeros((C + 1, 2), np.int32)
    idx_init_np[:, 0] = T
    Wr_np = np.ascontiguousarray(Wr.astype(np.float32))
    bf = ml_dtypes.bfloat16
    in_maps = []
    for c in range(E):
        oh = np.zeros((P, E), np.float32)
        oh[:, c] = 1.0
        in_maps.append({
            "xTr": xTr_np, "xbf": xbf_np, "Wr": Wr_np, "tri": tri_np,
            "ident": np.eye(P, dtype=ml_dtypes.bfloat16),
            "onehot": oh, "idx_init": idx_init_np,
            "w1": np.ascontiguousarray(W1[c].astype(bf)),
            "w3": np.ascontiguousarray(W3[c].astype(bf)),
            "w2": np.ascontiguousarray(W2[c].astype(bf)),
        })
    return in_maps


def host_combine(results, C=1280):
    out = np.zeros((T + 1, D), np.float32)
    for c in range(E):
        yT = results[c]["yT_out"]          # [D, C]
        idx = results[c]["idx_out"][:C, 0]  # [C]
        out[idx] += yT.T
    return out[:T]



_PROGRAM_CACHE = {}


def kernel(x, Wr, W1, W2, W3):
    C = 1280
    if "nc" not in _PROGRAM_CACHE:
        _PROGRAM_CACHE["nc"] = build_moe_program(1, "bf16", C)
    nc = _PROGRAM_CACHE["nc"]
    in_maps = host_prepare(np.asarray(x), np.asarray(Wr), np.asarray(W1),
                           np.asarray(W2), np.asarray(W3), "bf16", C)
    res = run_bass_kernel_spmd(nc, in_maps, list(range(E)))
    out = host_combine(res.results, C)
    return out.reshape(4, 1024, 1024).astype(np.float32)

